# revision 38
# baseline (speedup 1.0000x reference)
"""Fused multi-head attention kernel for Trainium2, SPMD over 8 NeuronCores.

Problem: nn_MultiHeadAttention (B=4, T=2048, C=512, H=8 heads, Dh=64).
  qkv = x @ W_attn + b_attn ; split q,k,v ; per-head softmax(q k^T / 8) v ;
  out = y @ W_out + b_out

Sharding: core c handles batch b = c//2 and heads hh*4..hh*4+3 (hh = c%2).
Each core computes a partial out-projection over its 4 heads' channels;
the host sums the two partials per batch and adds b_out.

Device-side layout is fully "transposed" (token axis on the free dim):
  xT [C, T] -> qT,kT [64h, T] (per head on partitions 0..63/64..127),
  v in natural [T, 256] layout padded with a ones column per head,
  scoresT [kpos, qpos] tiles -> exp on ScalarE -> AV matmuls give
  yT [d, qpos] with an extra row = softmax denominator (ones-column trick).
Softmax skips max-subtraction: scores ~ N(0,1), |s|max < ~10, safe in fp32.
Matmul inputs are bf16 (PSUM accumulation fp32); exp input fp32 from PSUM.
"""

import sys

if "/opt/trn_rl_repo" not in sys.path:
    sys.path.insert(0, "/opt/trn_rl_repo")

import numpy as np
import ml_dtypes

B, T_FULL, C = 4, 2048, 512
H, DH = 8, 64
HPC = 4  # heads per core
N_CORES = 8

_prog_cache = {}


def build_nc(T=T_FULL):
    import concourse.bass as bass
    import concourse.tile as tile
    from concourse import bacc, mybir
    from concourse.bass import ts

    f32 = mybir.dt.float32
    bf16 = mybir.dt.bfloat16
    # attention-probability dtype: fp16 (11-bit mantissa) is ~16x more precise
    # than bf16 for exp outputs, same 1 cyc/row PE rate; exp(s/8 - 2) keeps the
    # largest value ~e^6 even for outlier scores, far from fp16's 65504 max.
    f16 = mybir.dt.float16
    EXP_SHIFT = -2.0

    KT = T // 128         # kpos chunks
    NQ = max(1, T // 512) # q tiles of 512
    QW = min(T, 512)      # q tile width
    CH = HPC * DH         # 256 channels per core per q/k/v

    def pbcast(ap, nparts):
        """Partition-broadcast a 1-D (free-only) AP to [nparts, ...] for DMA."""
        return bass.AP(
            tensor=ap.tensor, offset=ap.offset, ap=[[0, nparts]] + list(ap.ap)
        )

    # Bacc (not raw Bass): its finalize() runs move_matmul_waits_to_ldweights +
    # generate_event_semaphores, legalizing the TRN2 1-wait-per-instruction limit.
    nc = bacc.Bacc("TRN2")

    xT = nc.dram_tensor("xT", [C, T], bf16, kind="ExternalInput")
    wq = nc.dram_tensor("wq", [C, CH], bf16, kind="ExternalInput")
    wk = nc.dram_tensor("wk", [C, CH], bf16, kind="ExternalInput")
    wv = nc.dram_tensor("wv", [C, CH], bf16, kind="ExternalInput")
    bq = nc.dram_tensor("bq", [CH], f32, kind="ExternalInput")
    bk = nc.dram_tensor("bk", [CH], f32, kind="ExternalInput")
    bv = nc.dram_tensor("bv", [CH], f32, kind="ExternalInput")
    wo = nc.dram_tensor("wo", [2, 128, C], bf16, kind="ExternalInput")
    out = nc.dram_tensor("out", [T, C], f32, kind="ExternalOutput")

    with tile.TileContext(nc) as tc:
        with (
            tc.tile_pool(name="consts", bufs=1) as consts,
            tc.tile_pool(name="ps_sc", bufs=1, space="PSUM") as ps_sc,
            tc.tile_pool(name="ps512", bufs=4, space="PSUM") as ps512,
            tc.tile_pool(name="expp", bufs=6) as expp,
            tc.tile_pool(name="rsm", bufs=4) as rsm,
            tc.tile_pool(name="rbp", bufs=4) as rbp,
            tc.tile_pool(name="outp", bufs=3) as outp,
        ):
            # ---- constant loads ----
            xT_sb = consts.tile([128, 4, T], bf16)
            nc.sync.dma_start(xT_sb[:], xT[:, :].rearrange("(ko p) t -> p ko t", p=128))
            wq_sb = consts.tile([128, 4, CH], bf16)
            nc.sync.dma_start(wq_sb[:], wq[:, :].rearrange("(ko p) m -> p ko m", p=128))
            wk_sb = consts.tile([128, 4, CH], bf16)
            nc.sync.dma_start(wk_sb[:], wk[:, :].rearrange("(ko p) m -> p ko m", p=128))
            wv_sb = consts.tile([128, 4, CH], bf16)
            nc.sync.dma_start(wv_sb[:], wv[:, :].rearrange("(ko p) m -> p ko m", p=128))
            # per-head rows at partitions 0-63 (mixed-row-base matmul
            # accumulation is broken on HW, so out-proj runs per-head base-0)
            wo_sb = consts.tile([64, HPC, C], bf16)
            nc.sync.dma_start(
                wo_sb[:], wo[:, :, :].rearrange("pr (h2 p) n -> p (pr h2) n", p=64)
            )
            bq_sb = consts.tile([128, 2], f32)
            nc.sync.dma_start(bq_sb[:], bq[:].rearrange("(o p) -> p o", p=128))
            bk_sb = consts.tile([128, 2], f32)
            nc.sync.dma_start(bk_sb[:], bk[:].rearrange("(o p) -> p o", p=128))
            bv_sb = consts.tile([128, CH], f32)
            nc.sync.dma_start(bv_sb[:], pbcast(bv[:], 128))
            exp_bias = consts.tile([128, 1], f32)
            nc.vector.memset(exp_bias[:], EXP_SHIFT)
            # Pre-touch DMA-loaded tiles on DVE: tensor_scalar/tensor_tensor
            # instructions have too few sync-wait slots to wait on both a PE
            # semaphore and a DMA semaphore; a cheap DVE read here makes the
            # DVE clock observe the DMA completion so later ops need only the
            # PE wait (walrus NCC_INLA001 "Too many sync wait commands").
            touch = consts.tile([128, 8], f32)
            nc.vector.tensor_copy(out=touch[:, 0:2], in_=bq_sb[:])
            nc.vector.tensor_copy(out=touch[:, 2:4], in_=bk_sb[:])
            nc.vector.tensor_copy(out=touch[:, 4:5], in_=bv_sb[:, 0:1])
            # Same trick for the PE clock: a dummy ldweights per DMA-loaded
            # matmul input makes PE observe the DMA queues once, so real
            # matmuls never carry a DMA wait on top of their compute waits.
            nc.tensor.ldweights(xT_sb[:, 0, 0:128])
            nc.tensor.ldweights(wq_sb[:, 0, 0:128])
            nc.tensor.ldweights(wk_sb[:, 0, 0:128])
            nc.tensor.ldweights(wv_sb[:, 0, 0:128])
            nc.tensor.ldweights(wo_sb[:, 0, 0:128])
            # ones row for the K=1 broadcast matmul in the softmax division
            ones64 = consts.tile([1, DH], f16)
            nc.vector.memset(ones64[:], 1.0)

            # ---- computed tensors ----
            # qT / kT: per 128-channel group (2 heads each), [128, T] bf16
            qT = [
                consts.tile([128, T], bf16, tag=f"qT{m}", name=f"qT{m}")
                for m in range(2)
            ]
            kT = [
                consts.tile([128, T], bf16, tag=f"kT{m}", name=f"kT{m}")
                for m in range(2)
            ]
            # v (natural layout) padded with ones column: [128, KT, HPC, 65]
            v_ones = consts.tile([128, KT, HPC, DH + 1], f16)
            nc.vector.memset(v_ones[:, :, :, DH : DH + 1], 1.0)
            # yT per head: [64, T] bf16 (normalized attention output, transposed)
            yT = [
                consts.tile([64, T], bf16, tag=f"yT{h}", name=f"yT{h}")
                for h in range(HPC)
            ]

            # ---- Phase 1: QKV projection ----
            # q,k (transposed layout): group m first so head-0 scores can start early
            for m in range(2):
                for w_sb, b_sb, dst in ((wq_sb, bq_sb, qT[m]), (wk_sb, bk_sb, kT[m])):
                    for nt in range(NQ):
                        pt = ps512.tile([128, 512], f32, tag="mm512")
                        for kt in range(4):
                            nc.tensor.matmul(
                                pt[:, :QW],
                                w_sb[:, kt, m * 128 : (m + 1) * 128],
                                xT_sb[:, kt, ts(nt, QW)],
                                start=(kt == 0),
                                stop=(kt == 3),
                            )
                        nc.vector.tensor_scalar_add(
                            out=dst[:, ts(nt, QW)],
                            in0=pt[:, :QW],
                            scalar1=b_sb[:, m : m + 1],
                        )
            # v natural layout
            for g in range(KT):
                pt = ps512.tile([128, 512], f32, tag="mm512")
                for kt in range(4):
                    nc.tensor.matmul(
                        pt[:, :CH],
                        xT_sb[:, kt, ts(g, 128)],
                        wv_sb[:, kt, :],
                        start=(kt == 0),
                        stop=(kt == 3),
                    )
                nc.vector.tensor_add(
                    out=v_ones[:, g, :, 0:DH],
                    in0=pt[:, :CH].rearrange("p (h d) -> p h d", h=HPC),
                    in1=bv_sb[:].rearrange("p (h d) -> p h d", h=HPC),
                )

            # ---- Phase 2: attention ----
            # Head-PAIR processing: heads hA=2p (partitions 0-63) and hB=2p+1
            # (64-127) issue adjacent row-tiled matmuls that run concurrently
            # on the PE array, writing disjoint column ranges of one scores
            # PSUM tile [128, T]: cols [0, T/2) = hA's qpos half, [T/2, T) =
            # hB's same qpos half. Each pair is covered in 2 "qh" phases.
            # AV matmuls lag 2 steps behind scores/exp (lag-2 pipeline).
            # bank-disjointness of the concurrent head-pair matmuls requires
            # each head's column range to cover whole PSUM banks (>=512 f32)
            assert T >= 1024, "pair-packed scores need T/2 >= 512 (PSUM bank)"
            HW2 = T // 2              # qpos width per head per scores tile
            QW2 = min(512, HW2)       # AV / division chunk width
            NQS = HW2 // QW2          # AV chains per head per phase
            expT = {}                 # step -> sbuf tile [128, T]
            av_ps = {}                # (h, qs) -> psum tile
            pending_div = []          # deferred division finishers
            NPH = 2 * 2               # pairs * qh phases
            NSTEP = NPH * KT

            def decode(s):
                ph, g = divmod(s, KT)
                p, qh = divmod(ph, 2)
                return p, qh, g

            # AV-step retiming: default lag 2 behind scores/exp, but the first
            # steps of each phase are deferred (g0@+4, g1@+5, g2+g3@+6,
            # g4+g5@+7, then steady lag-2) so the previous phase's divisions
            # (popped 2 per step at +2/+3) release all four AV PSUM slots
            # before this phase allocates its four.
            av_sched = {}
            for _ph in range(NPH):
                for _g in range(KT):
                    _aq = _ph * KT + _g
                    if _g == 0:
                        _run = _ph * KT + 4
                    elif _g == 1:
                        _run = _ph * KT + 5
                    elif _g in (2, 3):
                        _run = _ph * KT + 6
                    elif _g in (4, 5):
                        _run = _ph * KT + 7
                    else:
                        _run = _aq + 2
                    av_sched.setdefault(_run, []).append(_aq)

            def av_step(s):
                p, qh, g = decode(s)
                for h2 in range(2):
                    h = 2 * p + h2
                    for qs in range(NQS):
                        if g == 0:
                            av_ps[(h, qs)] = ps512.tile(
                                [128, 512], f32, tag="mm512", name=f"av_{s}_{h2}_{qs}"
                            )
                        nc.tensor.matmul(
                            av_ps[(h, qs)][: DH + 1, :QW2],
                            v_ones[:, g, h, :],
                            expT[s][:, h2 * HW2 + qs * QW2 : h2 * HW2 + (qs + 1) * QW2],
                            start=(g == 0),
                            stop=(g == KT - 1),
                        )
                if g == KT - 1:
                    emit_divs(p, qh)

            def emit_divs(p, qh):
                # Reciprocals now (DVE, off critical path); the PE broadcast +
                # normalize of each chain is deferred one per subsequent step
                # so the PE never stalls on a reciprocal.
                for h2 in range(2):
                    h = 2 * p + h2
                    for qs in range(NQS):
                        av = av_ps.pop((h, qs))
                        r16 = rsm.tile([1, 512], f16, tag="r16", name=f"r16_{h}_{qh}_{qs}")
                        with nc.allow_low_precision(reason="softmax recip in f16"):
                            nc.vector.reciprocal(
                                out=r16[:, :QW2], in_=av[DH : DH + 1, :QW2]
                            )
                        pending_div.append((h, qh, qs, r16, av))

            def finish_div(h, qh, qs, r16, av):
                # broadcast 1/denom across partitions via K=1 matmul into the
                # unused partitions 64..127 of the AV tile, then normalize.
                nc.tensor.matmul(
                    av[64:128, :QW2],
                    ones64[:],
                    r16[:, :QW2],
                    start=True,
                    stop=True,
                    tile_position=(0, 64),
                )
                rb = rbp.tile([64, 512], f32)
                nc.vector.tensor_copy(out=rb[:, :QW2], in_=av[64:128, :QW2])
                col = qh * HW2 + qs * QW2
                nc.vector.tensor_mul(
                    out=yT[h][:, col : col + QW2],
                    in0=av[0:DH, :QW2],
                    in1=rb[:, :QW2],
                )

            for s in range(NSTEP):
                p, qh, g = decode(s)
                for _ in range(2):
                    if pending_div:
                        finish_div(*pending_div.pop(0))
                for aq in av_sched.pop(s, []):
                    av_step(aq)
                # head A's scores first, then exp on A's half while head B's
                # scores run — bank-level PSUM deps let exp(s) overlap
                # scores(s) and scores(s+1) overlap exp(s) half-by-half.
                spt = ps_sc.tile([128, T], f32)
                et = expp.tile([128, T], f16)
                for h2 in range(2):
                    for qs in range(NQS):
                        nc.tensor.matmul(
                            spt[:, h2 * HW2 + qs * QW2 : h2 * HW2 + (qs + 1) * QW2],
                            kT[p][h2 * 64 : h2 * 64 + 64, ts(g, 128)],
                            qT[p][
                                h2 * 64 : h2 * 64 + 64,
                                qh * HW2 + qs * QW2 : qh * HW2 + (qs + 1) * QW2,
                            ],
                            start=True,
                            stop=True,
                        )
                    nc.scalar.activation(
                        out=et[:, h2 * HW2 : (h2 + 1) * HW2],
                        in_=spt[:, h2 * HW2 : (h2 + 1) * HW2],
                        func=mybir.ActivationFunctionType.Exp,
                        bias=exp_bias[:],
                        scale=1.0 / 8.0,
                    )
                expT[s] = et
            for s in sorted(av_sched):
                for _ in range(2):
                    if pending_div:
                        finish_div(*pending_div.pop(0))
                for aq in av_sched[s]:
                    av_step(aq)
            av_sched.clear()
            while pending_div:
                finish_div(*pending_div.pop(0))

            # ---- Phase 3: output projection (partial, this core's heads) ----
            for mq in range(T // 128):
                opt = ps512.tile([128, 512], f32, tag="mm512")
                for h in range(HPC):
                    nc.tensor.matmul(
                        opt[:],
                        yT[h][:, ts(mq, 128)],
                        wo_sb[:, h, :],
                        start=(h == 0),
                        stop=(h == HPC - 1),
                    )
                ot = outp.tile([128, 512], f32)
                nc.vector.tensor_copy(out=ot[:], in_=opt[:])
                nc.sync.dma_start(out[ts(mq, 128), :], ot[:])

    nc.finalize()
    return nc


def make_in_maps(x, W_attn, b_attn, W_out):
    """Shard full inputs across 8 cores: core c = (batch c//2, head-half c%2)."""
    bf = ml_dtypes.bfloat16
    in_maps = []
    for c in range(N_CORES):
        b, hh = divmod(c, 2)
        sl = slice(hh * HPC * DH, (hh + 1) * HPC * DH)  # channel slice (256)
        in_maps.append(
            {
                "xT": np.ascontiguousarray(x[b].T).astype(bf),
                "wq": W_attn[:, 0 * C :][:, sl].astype(bf),
                "wk": W_attn[:, 1 * C :][:, sl].astype(bf),
                "wv": W_attn[:, 2 * C :][:, sl].astype(bf),
                "bq": np.ascontiguousarray(b_attn[0 * C :][sl], dtype=np.float32),
                "bk": np.ascontiguousarray(b_attn[1 * C :][sl], dtype=np.float32),
                "bv": np.ascontiguousarray(b_attn[2 * C :][sl], dtype=np.float32),
                "wo": np.ascontiguousarray(
                    W_out[sl, :].reshape(2, 128, C)
                ).astype(bf),
            }
        )
    return in_maps


def kernel(x, W_attn, b_attn, W_out, b_out, _trace=False):
    from concourse.bass_utils import run_bass_kernel_spmd

    x = np.asarray(x, dtype=np.float32)
    W_attn = np.asarray(W_attn, dtype=np.float32)
    b_attn = np.asarray(b_attn, dtype=np.float32)
    W_out = np.asarray(W_out, dtype=np.float32)
    b_out = np.asarray(b_out, dtype=np.float32)

    key = T_FULL
    if key not in _prog_cache:
        _prog_cache[key] = build_nc(T_FULL)
    nc = _prog_cache[key]

    in_maps = make_in_maps(x, W_attn, b_attn, W_out)
    res = run_bass_kernel_spmd(nc, in_maps, list(range(N_CORES)), trace=_trace)

    out = np.empty((B, T_FULL, C), dtype=np.float32)
    for b in range(B):
        out[b] = res.results[2 * b]["out"] + res.results[2 * b + 1]["out"] + b_out
    if _trace:
        kernel.last_exec_time_ns = res.exec_time_ns
        kernel.last_results = res
    return out
